# revision 26
# baseline (speedup 1.0000x reference)
# AttnPool1dWindow Trainium2 kernel.
# B=8, C=512, L=4096, kernel_size=16, stride=8, P=511.
# Data-parallel: one batch per NeuronCore across 8 cores.
import numpy as np

B, C, L = 8, 512, 4096
WIN, ST = 16, 8
P = 1 + (L - WIN) // ST          # 511
P4 = 512                          # padded window count (last window dummy)
NEG = -1.0e9
NLC = 8                           # l-chunks of 512 tokens
NCT = 4                           # c tiles of 128
NDT = 4                           # d tiles of 128
NTT = 32                          # token tiles of 128
NPT = 4                           # p tiles of 128
LPAD = 8 * 528                    # srow length (= s8 rows * padded cols)

_CACHE = {}


def _build_host_constants():
    """Constant matrices shared by all cores (data independent)."""
    import ml_dtypes
    bf16 = ml_dtypes.bfloat16
    # E0[delta][r, q] = 1 iff q == 16*delta + r//8        (delta 0..7)
    # E1[delta][r, q] = 1 iff q == 16*delta + r//8 - 1    (delta 0..8)
    r = np.arange(128)
    q = np.arange(128)
    e0 = np.zeros((8, 128, 128), np.float32)
    for d in range(8):
        e0[d] = (q[None, :] == 16 * d + r[:, None] // 8).astype(np.float32)
    e1 = np.zeros((9, 128, 128), np.float32)
    for d in range(9):
        e1[d] = (q[None, :] == 16 * d + r[:, None] // 8 - 1).astype(np.float32)
    e0s = np.ascontiguousarray(e0.transpose(1, 0, 2).reshape(128, 8 * 128)).astype(bf16)
    e1s = np.ascontiguousarray(e1.transpose(1, 0, 2).reshape(128, 9 * 128)).astype(bf16)
    eye = np.eye(32, dtype=np.float32)
    return e0s, e1s, eye


def _custom_ap(ap, dims, extra_offset=0):
    """Copy of `ap` with explicit [step, count] dims (element units).

    NOTE: keep at most ONE non-mergeable free dim beyond the partition dim —
    the DMA lowering mis-steps middle dims of deeper APs.
    """
    import bass_rust
    c = ap.copy()
    c.ap = bass_rust.VecI64Pair(dims)
    if extra_offset:
        c.offset = c.offset + extra_offset
    return c


def _build_bass():
    import concourse.bacc as bacc
    import concourse.mybir as mybir
    import concourse.tile as tile
    dt = mybir.dt
    f32, bf16 = dt.float32, dt.bfloat16
    AF = mybir.ActivationFunctionType
    ALU = mybir.AluOpType

    nc = bacc.Bacc("TRN2", target_bir_lowering=False, debug=False, num_devices=8)

    # ---- DRAM I/O (per core) ----
    xbf_d = nc.declare_dram_parameter("xbf", [C, L], bf16, isOutput=False)
    xt_d = nc.declare_dram_parameter("xt", [L, C], bf16, isOutput=False)
    wt_d = nc.declare_dram_parameter("wtt", [C, C], bf16, isOutput=False)     # W^T [c,d]
    vemb_d = nc.declare_dram_parameter("vemb", [C, 64], bf16, isOutput=False)
    bcol_d = nc.declare_dram_parameter("bcol", [128, NDT], f32, isOutput=False)
    negm_d = nc.declare_dram_parameter("negm", [P4, WIN], f32, isOutput=False)
    notm_d = nc.declare_dram_parameter("notm", [P4, WIN], f32, isOutput=False)
    e0_d = nc.declare_dram_parameter("e0", [128, 8 * 128], bf16, isOutput=False)
    e1_d = nc.declare_dram_parameter("e1", [128, 9 * 128], bf16, isOutput=False)
    eye_d = nc.declare_dram_parameter("eye", [32, 32], f32, isOutput=False)
    out_d = nc.declare_dram_parameter("outt", [P4, C], f32, isOutput=True)    # out^T

    with tile.TileContext(nc) as tc:
        with (
            tc.tile_pool(name="big", bufs=1) as big,
            tc.tile_pool(name="hx", bufs=4) as hx,
            tc.tile_pool(name="smk", bufs=4) as smk,
            tc.tile_pool(name="outs", bufs=2) as outs,
            tc.tile_pool(name="rows", bufs=1, space="DRAM") as rows,
            tc.tile_pool(name="ypsum", bufs=2, space="PSUM") as ypsum,
            tc.tile_pool(name="spsum", bufs=1, space="PSUM") as spsum,
            tc.tile_pool(name="upsum", bufs=1, space="PSUM") as upsum,
            tc.tile_pool(name="opsum", bufs=2, space="PSUM") as opsum,
        ):
            # ---- resident SBUF tensors ----
            xbf = big.tile([128, NCT, L], bf16, tag="xbf")
            xt = big.tile([128, NTT, C], bf16, tag="xt")
            wtt = big.tile([128, NCT, C], bf16, tag="wtt")
            vemb = big.tile([128, NDT, 64], bf16, tag="vemb")
            bcol = big.tile([128, NDT], f32, tag="bcol")
            negm = big.tile([128, NPT, WIN], f32, tag="negm")
            notm = big.tile([128, NPT, WIN], f32, tag="notm")
            e0 = big.tile([128, 8 * 128], bf16, tag="e0")
            e1 = big.tile([128, 9 * 128], bf16, tag="e1")
            eye = big.tile([32, 32], f32, tag="eye")
            zt0 = big.tile([128, NTT, 512], bf16, tag="zt0")
            zt1 = big.tile([128, NTT, 512], bf16, tag="zt1")
            s8A = big.tile([5, 512], f32, tag="s8A")
            s8B = big.tile([3, 512], f32, tag="s8B")
            wtm = big.tile([128, NPT, WIN], f32, tag="wtm")
            u0chkA = big.tile([16, 128], f32, tag="u0chkA")
            u0chkB = big.tile([16, 128], f32, tag="u0chkB")
            u1chkA = big.tile([17, 128], f32, tag="u1chkA")
            u1chkB = big.tile([16, 128], f32, tag="u1chkB")
            u1chkbA = big.tile([17, 128], f32, tag="u1chkbA")
            u1chkbB = big.tile([16, 128], f32, tag="u1chkbB")
            ucol = big.tile([128, 64], f32, tag="ucol")
            sw_all = big.tile([128, NPT, WIN], f32, tag="sw_all")

            # DRAM scratch rows (token-ordered)
            srow = rows.tile([1, LPAD], f32, tag="srow")
            u0row = rows.tile([1, LPAD], f32, tag="u0row")
            u1row = rows.tile([1, LPAD], f32, tag="u1row")

            # ---- load inputs (2D APs only) ----
            for ci in range(NCT):
                nc.sync.dma_start(out=xbf[:, ci, :], in_=xbf_d[128 * ci:128 * ci + 128, :])
                nc.sync.dma_start(out=wtt[:, ci, :], in_=wt_d[128 * ci:128 * ci + 128, :])
            for di in range(NDT):
                nc.sync.dma_start(out=vemb[:, di, :], in_=vemb_d[128 * di:128 * di + 128, :])
            nc.sync.dma_start(out=bcol[:, :], in_=bcol_d[:, :])
            nc.sync.dma_start(
                out=negm[:, :, :],
                in_=_custom_ap(negm_d[:], [[WIN, 128], [128 * WIN, NPT], [1, WIN]]),
            )
            nc.sync.dma_start(
                out=notm[:, :, :],
                in_=_custom_ap(notm_d[:], [[WIN, 128], [128 * WIN, NPT], [1, WIN]]),
            )
            nc.sync.dma_start(out=eye[:, :], in_=eye_d[:, :])

            # zero u1row once: boundary chunk reads of not-yet-written tails
            # must see finite values (they hit zero-weight rows of E1).
            zrow = big.tile([1, LPAD], f32, tag="zrow")
            nc.vector.memset(zrow[0:1, :], 0.0)
            nc.sync.dma_start(out=u1row[0:1, :], in_=zrow[0:1, :])

            up_pool = upsum
            spA = spsum.tile([8, 512], f32, tag="SPA")
            spB = spsum.tile([8, 512], f32, tag="SPB")

            def phase1_chunk(li, sptile, first, last):
                for di in range(NDT):
                    yp = ypsum.tile([128, 512], f32, tag="Y")
                    for ci in range(NCT):
                        nc.tensor.matmul(
                            yp[:, :],
                            wtt[:, ci, 128 * di:128 * di + 128],
                            xbf[:, ci, 512 * li:512 * li + 512],
                            start=(ci == 0),
                            stop=(ci == NCT - 1),
                        )
                    h = hx.tile([128, 512], bf16, tag="H")
                    nc.scalar.activation(h[:, :], yp[:, :], AF.Tanh,
                                         bias=bcol[:, di:di + 1])
                    nc.tensor.matmul(
                        sptile[0:8, :],
                        vemb[:, di, 8 * li:8 * li + 8],
                        h[:, :],
                        start=(first and di == 0),
                        stop=(last and di == NDT - 1),
                    )

            def tail(half):
                # half 0: p-tiles 0..2 (tokens 0..2063); half 1: p-tiles 2..4
                ks = (0, 1) if half == 0 else (2, 3)
                if half == 0:
                    nc.scalar.copy(s8A[0:5, :], spA[0:5, :])
                    nc.sync.dma_start(out=srow[0:1, 0:2560], in_=s8A[0:5, :])
                else:
                    nc.scalar.copy(s8B[0:3, :], spB[0:3, :])
                    nc.sync.dma_start(out=srow[0:1, 2560:L], in_=s8B[0:3, :])
                    nc.sync.dma_start(out=srow[0:1, L:LPAD], in_=zrow[0:1, 0:128])
                for k in ks:
                    nc.sync.dma_start(
                        out=sw_all[:, k, :],
                        in_=_custom_ap(srow[:], [[LPAD, 1], [ST, 128], [1, WIN]],
                                       1024 * k),
                    )
                for k in ks:
                    swm = smk.tile([128, WIN], f32, tag="swm")
                    nc.vector.tensor_add(swm[:, :], sw_all[:, k, :], negm[:, k, :])
                    mx = smk.tile([128, 1], f32, tag="mx")
                    nc.vector.tensor_reduce(
                        mx[:, :], swm[:, :], axis=mybir.AxisListType.X, op=ALU.max,
                    )
                    mxn = smk.tile([128, 1], f32, tag="mxn")
                    nc.vector.tensor_scalar_mul(mxn[:, :], mx[:, :], -1.0)
                    ek = smk.tile([128, WIN], f32, tag="ek")
                    den = smk.tile([128, 1], f32, tag="den")
                    nc.scalar.activation(ek[:, :], swm[:, :], AF.Exp,
                                         bias=mxn[:, :], accum_out=den[:, :])
                    rden = smk.tile([128, 1], f32, tag="rden")
                    nc.vector.reciprocal(rden[:, :], den[:, :])
                    ewn = smk.tile([128, WIN], f32, tag="ewn")
                    nc.vector.tensor_mul(ewn[:, :], ek[:, :], notm[:, k, :])
                    nc.vector.tensor_scalar_mul(wtm[:, k, :], ewn[:, :], rden[:, :])
                    nc.sync.dma_start(
                        out=_custom_ap(u0row[:], [[LPAD, 1], [ST, 128], [1, 8]],
                                       1024 * k),
                        in_=wtm[:, k, 0:8],
                    )
                    nc.sync.dma_start(
                        out=_custom_ap(u1row[:], [[LPAD, 1], [ST, 128], [1, 8]],
                                       1024 * k + 8),
                        in_=wtm[:, k, 8:16],
                    )
                if half == 0:
                    nc.sync.dma_start(out=u0chkA[:, :], in_=u0row[0:1, 0:2048])
                    nc.sync.dma_start(out=u1chkA[:, :], in_=u1row[0:1, 0:2176])
                    nc.vector.tensor_copy(u1chkbA[:, :], u1chkA[:, :])
                    nc.vector.memset(u1chkbA[0:1, 0:8], 0.0)
                    upt = up_pool.tile([128, 33], f32, tag="UP")
                    nc.tensor.transpose(upt[:, 0:16], u0chkA[:, :], eye[0:16, 0:16])
                    nc.tensor.transpose(upt[:, 16:33], u1chkbA[:, :], eye[0:17, 0:17])
                    nc.vector.tensor_copy(ucol[:, 0:16], upt[:, 0:16])
                    nc.vector.tensor_copy(ucol[:, 32:49], upt[:, 16:33])
                    trng = range(0, 17)
                else:
                    nc.sync.dma_start(out=u0chkB[:, :], in_=u0row[0:1, 2048:L])
                    nc.sync.dma_start(out=u1chkB[:, :], in_=u1row[0:1, 2048:L])
                    nc.vector.tensor_copy(u1chkbB[:, :], u1chkB[:, :])
                    upt = up_pool.tile([128, 33], f32, tag="UP")
                    nc.tensor.transpose(upt[:, 0:16], u0chkB[:, :], eye[0:16, 0:16])
                    nc.tensor.transpose(upt[:, 16:32], u1chkbB[:, :], eye[0:16, 0:16])
                    nc.vector.tensor_copy(ucol[:, 16:32], upt[:, 0:16])
                    nc.vector.tensor_copy(ucol[:, 48:64], upt[:, 16:32])
                    trng = range(16, NTT)
                for t in trng:
                    if t < 32:
                        nc.vector.tensor_scalar_mul(zt0[:, t, :], xt[:, t, :],
                                                    ucol[:, t:t + 1])
                    nc.vector.tensor_scalar_mul(zt1[:, t, :], xt[:, t, :],
                                                ucol[:, 32 + t:32 + t + 1])
                for k in ks:
                    op = opsum.tile([128, 512], f32, tag="OP")
                    for d in range(8):
                        nc.tensor.matmul(
                            op[:, :], e0[:, 128 * d:128 * d + 128],
                            zt0[:, 8 * k + d, :],
                            start=(d == 0), stop=False,
                        )
                    for d in range(9):
                        t = 8 * k + d
                        if t >= NTT:
                            continue
                        nc.tensor.matmul(
                            op[:, :], e1[:, 128 * d:128 * d + 128], zt1[:, t, :],
                            start=False, stop=(d == 8 or t == NTT - 1),
                        )
                    ot = outs.tile([128, 512], f32, tag="OT")
                    nc.vector.tensor_copy(ot[:, :], op[:, :])
                    nc.sync.dma_start(out=out_d[128 * k:128 * k + 128, :],
                                        in_=ot[:, :])

            # ---- pipelined schedule: tail(0) overlaps phase-1 chunks 5..7 ----
            for li in range(5):
                phase1_chunk(li, spA, first=(li == 0), last=(li == 4))
            nc.sync.dma_start(out=e0[:, :], in_=e0_d[:, :])
            nc.sync.dma_start(out=e1[:, :], in_=e1_d[:, :])
            for tt in range(0, NTT, 8):
                nc.sync.dma_start(
                    out=xt[:, tt:tt + 8, :],
                    in_=_custom_ap(xt_d[:], [[C, 1], [C, 128], [128 * C, 8], [1, C]],
                                   128 * tt * C),
                )
            tail(0)
            for li in range(5, 8):
                phase1_chunk(li, spB, first=(li == 5), last=(li == 7))
            tail(1)
    nc.compile()
    return nc


def _prep_inputs(x, mask, W, b_, v):
    """Host-side shard prep: core i gets batch i."""
    import ml_dtypes
    bf16 = ml_dtypes.bfloat16
    e0s, e1s, eye = _build_host_constants()

    wtt = np.ascontiguousarray(W.T).astype(bf16)      # [c, d]
    vemb = np.zeros((C, 64), np.float32)
    for li in range(8):
        row = li if li < 5 else li - 5
        vemb[:, li * 8 + row] = v
    vemb = vemb.astype(bf16)
    bcol = np.ascontiguousarray(b_.reshape(NDT, 128).T).astype(np.float32)  # [128, 4]

    pidx = np.arange(P4)
    widx = np.arange(WIN)
    tok = pidx[:, None] * ST + widx[None, :]          # [P4, 16]
    valid = tok < L

    maps = []
    for bi in range(B):
        mw = np.ones((P4, WIN), bool)
        mw[valid] = mask[bi][tok[valid]]
        xb = x[bi]                                    # [C, L] fp32
        maps.append({
            "xbf": xb.astype(bf16),
            "xt": np.ascontiguousarray(xb.T).astype(bf16),
            "wtt": wtt,
            "vemb": vemb,
            "bcol": bcol,
            "negm": np.where(mw, np.float32(NEG), np.float32(0.0)),
            "notm": np.where(mw, np.float32(0.0), np.float32(1.0)),
            "e0": e0s,
            "e1": e1s,
            "eye": eye,
        })
    return maps


def kernel(x, mask, W, b, v):
    x = np.asarray(x, np.float32)
    mask = np.asarray(mask, bool)
    W = np.asarray(W, np.float32)
    b = np.asarray(b, np.float32)
    v = np.asarray(v, np.float32)

    from concourse.bass_utils import run_bass_kernel_spmd
    if "nc" not in _CACHE:
        _CACHE["nc"] = _build_bass()
    nc = _CACHE["nc"]

    in_maps = _prep_inputs(x, mask, W, b, v)
    res = run_bass_kernel_spmd(nc, in_maps, core_ids=list(range(8)))
    out = np.zeros((B, C, P), np.float32)
    for bi in range(B):
        outt = np.asarray(res.results[bi]["outt"], np.float32)   # [p, c]
        out[bi] = outt[:P].T
    return out


if __name__ == "__main__":
    import reference
    inputs = reference.setup_inputs()
    got = kernel(**{k: np.asarray(vv) for k, vv in inputs.items()})
    exp = np.asarray(reference.reference(**inputs))
    err = np.abs(got - exp).max() / np.abs(exp).max()
    print("scale-rel max err:", err)
